# revision 1
# baseline (speedup 1.0000x reference)
"""AlexNet-style CNN forward pass on 8 Trainium2 NeuronCores.

Strategy:
  - Convs data-parallel: batch 256 -> 32 per core, channels on partitions,
    conv = sum of shifted matmuls over kernel offsets (weights replicated).
  - conv1 (cin=3) uses host-packed im2col rows (3 dy-offsets x 11 dx x 3 ch
    + ones row for fused bias -> K=100) so the PE array is well utilized.
  - conv2 uses an x-shifted duplicate of its input (K=128 = 2 dx-offsets
    x 64 ch) to fill the contraction dim.
  - FC layers model-parallel: each core owns 512 rows of fc1/fc2 and 512
    K-columns of fc3; activations are AllGathered between layers, fc3
    partials AllReduced.  This cuts per-core FC weight DMA 8x.
  - Matmuls run as float32r (relaxed fp32, ~4x faster than strict fp32);
    everything else fp32.
"""

import numpy as np

import concourse.bass as bass
import concourse.mybir as mybir
import concourse.tile as tile
from concourse import bacc
from concourse.bass_utils import run_bass_kernel_spmd

N_CORES = 8
B = 256
BC = B // N_CORES  # 32 images per core

F32 = mybir.dt.float32
F32R = mybir.dt.float32r
RELU = mybir.ActivationFunctionType.Relu
IDENT = mybir.ActivationFunctionType.Identity


def _r(ap):
    return ap.bitcast(F32R)


def _emit(nc, tc, t, yout):
    """Emit the whole network. t: dict name -> DRAM AP."""
    sync = nc.sync
    act = nc.scalar
    dve = nc.vector
    pool_e = nc.gpsimd

    psum = tc.alloc_tile_pool(name="psum", bufs=6, space="PSUM")
    scr = tc.alloc_tile_pool(name="scr", bufs=1, side="left")
    dram = tc.alloc_tile_pool(name="dram", bufs=1, space="DRAM")

    # ---------------- phase pools (queue alloc mode handles overlap) ----
    p_w12 = tc.alloc_tile_pool(name="p_w12", bufs=1, side="left")
    p_x2s = tc.alloc_tile_pool(name="p_x2s", bufs=1, side="left")
    p_x13 = tc.alloc_tile_pool(name="p_x13", bufs=2, side="left")

    # conv1+conv2 weights (host arrays already in SBUF layout)
    lw1 = p_w12.tile([100, 4 * 64], F32R)
    sync.dma_start(lw1[:], t["lw1"][:])
    lw2 = p_w12.tile([128, 7 * 4 * 192], F32R)
    sync.dma_start(lw2[:], t["lw2"][:])
    lb2 = p_w12.tile([128, 2], F32)
    sync.dma_start(lb2[:], t["lb2"][:])

    # conv2 input: [128, BC, 22, 23]; rows 0:64 ch c at x, rows 64:128 ch c at x+1
    X2s = p_x2s.tile([128, BC * 22 * 23], F32R)
    pool_e.memset(X2s[:].bitcast(F32), 0.0)

    def x2v(p0, p1, b0, nb, y0, ny, x0, nx):
        return X2s[p0:p1].rearrange("p (b y x) -> p b y x", b=BC, y=22, x=23)[
            :, b0:b0 + nb, y0:y0 + ny, x0:x0 + nx]

    # ---------------- conv1 + pool1 ----------------
    _sid = nc.enter_named_scope("L1_conv1", False)[0]
    for bg in range(4):  # groups of 8 images
        xt = p_x13.tile([100, 8 * 41 * 32], F32R, tag="x13")
        sync.dma_start(xt[:], t["x13"][bg])
        xtv = xt.rearrange("k (b y x) -> k b y x", b=8, y=41, x=32)
        for bl in range(8):
            b = bg * 8 + bl
            for h in range(2):  # vertical half of the 32x32 output
                ps = psum.tile([64, 512], F32, tag="ps")
                psv = ps.rearrange("m (y x) -> m y x", y=16, x=32)
                for pi, p in enumerate((0, 3, 6, 9)):
                    nc.tensor.matmul(
                        ps[:],
                        _r(lw1[:, pi * 64:(pi + 1) * 64]),
                        _r(xtv[:, bl, h * 16 + p:h * 16 + p + 16, :]),
                        start=(pi == 0), stop=(pi == 3),
                    )
                # evict+relu (bias came in via the ones-row), then 2x2 maxpool
                s1 = scr.tile([128, 512], F32, tag="ev", bufs=3)
                act.activation(s1[0:64, :], ps[:], RELU)
                s1v = s1[0:64, :].rearrange("m (y x) -> m y x", y=16, x=32)
                m1 = scr.tile([64, 128], F32, tag="m1", bufs=2)
                m2 = scr.tile([64, 128], F32, tag="m2", bufs=2)
                dve.tensor_max(m1[:], s1v[:, 0::2, 0::2], s1v[:, 0::2, 1::2])
                dve.tensor_max(m2[:], s1v[:, 1::2, 0::2], s1v[:, 1::2, 1::2])
                y0 = h * 8 + 3
                dve.tensor_max(
                    x2v(0, 64, b, 1, y0, 8, 3, 16)[:, 0],
                    m1.rearrange("m (y x) -> m y x", y=8, x=16),
                    m2.rearrange("m (y x) -> m y x", y=8, x=16))
        # duplicate this image-group into the x+1-shifted partition block
        # (engines cannot shift partitions; DMA can)
        sync.dma_start(x2v(64, 128, bg * 8, 8, 0, 22, 0, 22),
                       x2v(0, 64, bg * 8, 8, 0, 22, 1, 22))
    p_x13.release()
    nc.leave_named_scope("L1_conv1", _sid, False)

    # conv3 weights (prefetch during conv2) + conv3 input buffers
    p_w3 = tc.alloc_tile_pool(name="p_w3", bufs=1, side="right")
    p_x3 = tc.alloc_tile_pool(name="p_x3", bufs=1, side="right")
    lw3 = p_w3.tile([128, 14592], F32R)
    sync.dma_start(lw3[:], t["lw3"][:])
    lb3 = p_w3.tile([128, 3], F32)
    sync.dma_start(lb3[:], t["lb3"][:])
    X3a = p_x3.tile([128, BC * 12 * 12], F32R)
    # X3b rows 64:128 duplicate rows 0:64 so kc1 matmuls can run at
    # lhsT base_partition 64 (lw3 packs two kernel offsets per column block)
    X3b = p_x3.tile([128, BC * 12 * 12], F32R)
    pool_e.memset(X3a[:].bitcast(F32), 0.0)
    pool_e.memset(X3b[:].bitcast(F32), 0.0)

    def x3v(xab, p0, p1, b0, nb, y0, ny, x0, nx):
        return xab[p0:p1].rearrange("p (b y x) -> p b y x", b=BC, y=12, x=12)[
            :, b0:b0 + nb, y0:y0 + ny, x0:x0 + nx]

    # ---------------- conv2 + pool2 ----------------
    _sid = nc.enter_named_scope("L2_conv2", False)[0]
    lw2v = lw2.rearrange("k (a j m) -> k a j m", a=7, j=4, m=192)
    for nt in range(16):  # pairs of images
        for mc in range(2):
            M = 128 if mc == 0 else 64
            ps = psum.tile([M, 512], F32, tag="ps")
            first = True
            for dy in range(7):
                for j in range(4):
                    K = 128 if j < 3 else 64
                    xoff = 2 * j if j < 3 else 6
                    nc.tensor.matmul(
                        ps[:],
                        _r(lw2v[0:K, dy, j, mc * 128:mc * 128 + M]),
                        _r(x2v(0, K, nt * 2, 2, dy, 16, xoff, 16)),
                        start=first, stop=(dy == 6 and j == 3),
                    )
                    first = False
            s2 = scr.tile([128, 512], F32, tag="ev", bufs=3)
            act.activation(s2[:M], ps[:], RELU, bias=lb2[0:M, mc:mc + 1])
            s2v = s2.rearrange("m (b y x) -> m b y x", b=2, y=16, x=16)
            m1 = scr.tile([128, 128], F32, tag="m1", bufs=2)
            m2 = scr.tile([128, 128], F32, tag="m2", bufs=2)
            dve.tensor_max(m1[:M], s2v[:M, :, 0::2, 0::2], s2v[:M, :, 0::2, 1::2])
            dve.tensor_max(m2[:M], s2v[:M, :, 1::2, 0::2], s2v[:M, :, 1::2, 1::2])
            m1v = m1.rearrange("m (b y x) -> m b y x", b=2, y=8, x=8)
            m2v = m2.rearrange("m (b y x) -> m b y x", b=2, y=8, x=8)
            if mc == 0:
                dve.tensor_max(x3v(X3a, 0, 128, nt * 2, 2, 2, 8, 2, 8), m1v[:], m2v[:])
            else:
                dve.tensor_max(x3v(X3b, 0, 64, nt * 2, 2, 2, 8, 2, 8), m1v[:64], m2v[:64])
    for g in range(4):  # duplicate X3b into partitions 64:128
        sync.dma_start(x3v(X3b, 64, 128, g * 8, 8, 0, 12, 0, 12),
                       x3v(X3b, 0, 64, g * 8, 8, 0, 12, 0, 12))
    nc.leave_named_scope("L2_conv2", _sid, False)
    p_x2s.release()
    p_w12.release()

    # conv4/5 weights (prefetch during conv3) + conv4 input buffers
    p_w45 = tc.alloc_tile_pool(name="p_w45", bufs=1, side="left")
    p_x4 = tc.alloc_tile_pool(name="p_x4", bufs=1, side="left")
    lw4 = p_w45.tile([128, 27 * 256], F32R)
    sync.dma_start(lw4[:], t["lw4"][:])
    lb4 = p_w45.tile([128, 2], F32)
    sync.dma_start(lb4[:], t["lb4"][:])
    lw5 = p_w45.tile([128, 18 * 256], F32R)
    sync.dma_start(lw5[:], t["lw5"][:])
    lb5 = p_w45.tile([128, 2], F32)
    sync.dma_start(lb5[:], t["lb5"][:])
    X4 = []
    for i in range(3):
        X4.append(p_x4.tile([128, BC * 10 * 10], F32R, name=f"X4_{i}"))
        pool_e.memset(X4[i][:].bitcast(F32), 0.0)

    def xv10(xab, p0, p1, b0, nb, y0, ny, x0, nx):
        return xab[p0:p1].rearrange("p (b y x) -> p b y x", b=BC, y=10, x=10)[
            :, b0:b0 + nb, y0:y0 + ny, x0:x0 + nx]

    _sid = nc.enter_named_scope("L3_conv3", False)[0]
    # ---------------- conv3 ----------------
    for nt in range(4):  # 8 images
        for mc in range(3):
            ps = psum.tile([128, 512], F32, tag="ps")
            first = True
            for dy in range(5):
                for dx in range(5):
                    blk = dy * 5 + dx
                    nc.tensor.matmul(
                        ps[:],
                        _r(lw3[0:128, blk * 384 + mc * 128:blk * 384 + mc * 128 + 128]),
                        _r(x3v(X3a, 0, 128, nt * 8, 8, dy, 8, dx, 8)),
                        start=first, stop=False,
                    )
                    first = False
                    po = 64 * (blk % 2)
                    co = 9600 + (blk // 2) * 384
                    nc.tensor.matmul(
                        ps[:],
                        _r(lw3[po:po + 64, co + mc * 128:co + mc * 128 + 128]),
                        _r(x3v(X3b, po, po + 64, nt * 8, 8, dy, 8, dx, 8)),
                        start=False, stop=(dy == 4 and dx == 4),
                    )
            act.activation(
                xv10(X4[mc], 0, 128, nt * 8, 8, 1, 8, 1, 8),
                ps.rearrange("m (b y x) -> m b y x", b=8, y=8, x=8),
                RELU, bias=lb3[:, mc:mc + 1])
    nc.leave_named_scope("L3_conv3", _sid, False)
    p_x3.release()
    p_w3.release()

    # fc1 weights (prefetch during conv4) + conv5 input buffers
    p_fw1 = tc.alloc_tile_pool(name="p_fw1", bufs=1, side="right")
    p_x5 = tc.alloc_tile_pool(name="p_x5", bufs=1, side="right")
    fw1 = p_fw1.tile([128, 32 * 512], F32R)
    sync.dma_start(fw1[:], t["fw1s"][:])
    fb1 = p_fw1.tile([128, 4], F32)
    sync.dma_start(fb1[:], t["fb1s"][:])
    X5 = []
    for i in range(2):
        X5.append(p_x5.tile([128, BC * 10 * 10], F32R, name=f"X5_{i}"))
        pool_e.memset(X5[i][:].bitcast(F32), 0.0)

    _sid = nc.enter_named_scope("L4_conv4", False)[0]
    # ---------------- conv4 ----------------
    lw4v = lw4.rearrange("k (o m) -> k o m", o=27)
    for nt in range(4):
        for mc in range(2):
            ps = psum.tile([128, 512], F32, tag="ps")
            first = True
            for dy in range(3):
                for dx in range(3):
                    for kc in range(3):
                        o = (dy * 3 + dx) * 3 + kc
                        nc.tensor.matmul(
                            ps[:],
                            _r(lw4v[:, o, mc * 128:mc * 128 + 128]),
                            _r(xv10(X4[kc], 0, 128, nt * 8, 8, dy, 8, dx, 8)),
                            start=first, stop=(o == 26),
                        )
                        first = False
            act.activation(
                xv10(X5[mc], 0, 128, nt * 8, 8, 1, 8, 1, 8),
                ps.rearrange("m (b y x) -> m b y x", b=8, y=8, x=8),
                RELU, bias=lb4[:, mc:mc + 1])
    nc.leave_named_scope("L4_conv4", _sid, False)
    p_x4.release()

    # pool5 output
    p_p5 = tc.alloc_tile_pool(name="p_p5", bufs=1, side="left")
    P5 = [p_p5.tile([128, BC * 16], F32R, name=f"P5_{i}") for i in range(2)]

    _sid = nc.enter_named_scope("L5_conv5", False)[0]
    # ---------------- conv5 + pool5 ----------------
    lw5v = lw5.rearrange("k (o m) -> k o m", o=18)
    for nt in range(4):
        for mc in range(2):
            ps = psum.tile([128, 512], F32, tag="ps")
            first = True
            for dy in range(3):
                for dx in range(3):
                    for kc in range(2):
                        o = (dy * 3 + dx) * 2 + kc
                        nc.tensor.matmul(
                            ps[:],
                            _r(lw5v[:, o, mc * 128:mc * 128 + 128]),
                            _r(xv10(X5[kc], 0, 128, nt * 8, 8, dy, 8, dx, 8)),
                            start=first, stop=(o == 17),
                        )
                        first = False
            s5 = scr.tile([128, 512], F32, tag="ev", bufs=3)
            act.activation(s5[:], ps[:], RELU, bias=lb5[:, mc:mc + 1])
            s5v = s5.rearrange("m (b y x) -> m b y x", b=8, y=8, x=8)
            m1 = scr.tile([128, 128], F32, tag="m1", bufs=2)
            m2 = scr.tile([128, 128], F32, tag="m2", bufs=2)
            dve.tensor_max(m1[:], s5v[:, :, 0::2, 0::2], s5v[:, :, 0::2, 1::2])
            dve.tensor_max(m2[:], s5v[:, :, 1::2, 0::2], s5v[:, :, 1::2, 1::2])
            p5v = P5[mc].rearrange("p (b y x) -> p b y x", b=BC, y=4, x=4)
            dve.tensor_max(
                p5v[:, nt * 8:nt * 8 + 8, :, :],
                m1.rearrange("m (b y x) -> m b y x", b=8, y=4, x=4),
                m2.rearrange("m (b y x) -> m b y x", b=8, y=4, x=4))
    nc.leave_named_scope("L5_conv5", _sid, False)
    # stage pool5 out to DRAM, then free conv-era pools (LIFO per side)
    cin5 = dram.tile([2, 128, BC * 16], F32R)
    sync.dma_start(cin5[0], P5[0][:])
    sync.dma_start(cin5[1], P5[1][:])
    p_x5.release()
    p_p5.release()
    p_w45.release()

    # fc2/fc3 weights (DMA overlaps the gather + fc1)
    p_fw2 = tc.alloc_tile_pool(name="p_fw2", bufs=1, side="left")
    fw2 = p_fw2.tile([128, 32 * 512], F32R)
    sync.dma_start(fw2[:], t["fw2s"][:])
    fb2 = p_fw2.tile([128, 4], F32)
    sync.dma_start(fb2[:], t["fb2s"][:])
    fw3 = p_fw2.tile([128, 4 * 100], F32R)
    sync.dma_start(fw3[:], t["fw3s"][:])
    fb3 = p_fw2.tile([100, 1], F32)
    sync.dma_start(fb3[:], t["fb3s"][:])

    _sid = nc.enter_named_scope("G1_gather", False)[0]
    # ---------------- AllGather pool5 -> fc input ----------------
    g1 = dram.tile([N_CORES, 2, 128, BC * 16], F32R)
    pool_e.collective_compute(
        "AllGather", mybir.AluOpType.bypass,
        replica_groups=[list(range(N_CORES))],
        ins=[cin5.opt()], outs=[g1.opt()])

    p_h1 = tc.alloc_tile_pool(name="p_h1", bufs=1, side="right")
    H1 = [p_h1.tile([128, N_CORES * BC * 16], F32R, name=f"H1_{i}") for i in range(2)]
    for cc in range(2):
        sync.dma_start(
            H1[cc].rearrange("c (r f) -> c r f", r=N_CORES),
            g1[:, cc].rearrange("r c f -> c r f"))

    nc.leave_named_scope("G1_gather", _sid, False)
    _sid = nc.enter_named_scope("F1_fc1", False)[0]
    # ---------------- fc1 (model-parallel over 512 outputs) ----------------
    p_f1 = tc.alloc_tile_pool(name="p_f1", bufs=1, side="left")
    F1 = p_f1.tile([128, 4 * B], F32R)
    fw1v = fw1.rearrange("k (y c m) -> k y c m", y=16, c=2, m=512)
    for mc in range(4):
        ps = psum.tile([128, B], F32, tag="ps")
        first = True
        for yx in range(16):
            for cc in range(2):
                rhs = H1[cc].rearrange("c (r b y) -> c y r b", r=N_CORES, b=BC, y=16)
                nc.tensor.matmul(
                    ps[:],
                    _r(fw1v[:, yx, cc, mc * 128:mc * 128 + 128]),
                    _r(rhs[:, yx]),
                    start=first, stop=(yx == 15 and cc == 1))
                first = False
        act.activation(F1[:, mc * B:(mc + 1) * B], ps[:], RELU, bias=fb1[:, mc:mc + 1])
    p_h1.release()
    p_fw1.release()

    nc.leave_named_scope("F1_fc1", _sid, False)
    _sid = nc.enter_named_scope("G2_gather", False)[0]
    # ---------------- AllGather fc1 ----------------
    cin6 = dram.tile([128, 4 * B], F32R)
    sync.dma_start(cin6[:], F1[:])
    g2 = dram.tile([N_CORES, 128, 4 * B], F32R)
    pool_e.collective_compute(
        "AllGather", mybir.AluOpType.bypass,
        replica_groups=[list(range(N_CORES))],
        ins=[cin6.opt()], outs=[g2.opt()])
    p_f1.release()

    p_h2 = tc.alloc_tile_pool(name="p_h2", bufs=1, side="right")
    H2 = p_h2.tile([128, N_CORES * 4 * B], F32R)
    sync.dma_start(
        H2.rearrange("c (r f) -> c r f", r=N_CORES),
        g2.rearrange("r c f -> c r f"))

    nc.leave_named_scope("G2_gather", _sid, False)
    _sid = nc.enter_named_scope("F2_fc2", False)[0]
    # ---------------- fc2 ----------------
    p_f2 = tc.alloc_tile_pool(name="p_f2", bufs=1, side="left")
    F2 = p_f2.tile([128, 4 * B], F32R)
    fw2v = fw2.rearrange("k (a m) -> k a m", a=32)
    for mc in range(4):
        ps = psum.tile([128, B], F32, tag="ps")
        for kc in range(32):
            nc.tensor.matmul(
                ps[:], _r(fw2v[:, kc, mc * 128:mc * 128 + 128]),
                _r(H2[:, kc * B:(kc + 1) * B]),
                start=(kc == 0), stop=(kc == 31))
        act.activation(F2[:, mc * B:(mc + 1) * B], ps[:], RELU, bias=fb2[:, mc:mc + 1])
    p_h2.release()

    nc.leave_named_scope("F2_fc2", _sid, False)
    _sid = nc.enter_named_scope("F3_fc3", False)[0]
    # ---------------- fc3 (partial over this core's 512 K) + AllReduce ----
    fw3v = fw3.rearrange("k (a m) -> k a m", a=4)
    ps = psum.tile([128, B], F32, tag="ps")
    for kc in range(4):
        nc.tensor.matmul(
            ps[0:100, :], _r(fw3v[:, kc, :]), _r(F2[:, kc * B:(kc + 1) * B]),
            start=(kc == 0), stop=(kc == 3))
    s3 = scr.tile([128, 512], F32, tag="ev", bufs=3)
    act.activation(s3[0:100, 0:B], ps[0:100, :], IDENT, bias=fb3[:])  # + fb3/8
    cin7 = dram.tile([100, B], F32)
    sync.dma_start(cin7[:], s3[0:100, 0:B])
    g3 = dram.tile([100, B], F32)
    pool_e.collective_compute(
        "AllReduce", mybir.AluOpType.add,
        replica_groups=[list(range(N_CORES))],
        ins=[cin7.opt()], outs=[g3.opt()])
    sync.dma_start(yout[:], g3[:])
    nc.leave_named_scope("F3_fc3", _sid, False)
    p_f2.release()
    p_fw2.release()

    scr.release()
    dram.release()
    psum.release()


# ---------------------------------------------------------------------------
# host-side input prep (numpy; all weight arrays already in SBUF layout)
# ---------------------------------------------------------------------------

def _prep_shared(w1, b1, w2, b2, w3, b3, w4, b4, w5, b5):
    f = np.float32
    # conv1: rows r = dyo*33 + dx*3 + c, row 99 = bias(ones); 4 passes dy=3p+dyo
    lw1 = np.zeros((100, 4 * 64), f)
    for p in range(4):
        for dyo in range(3):
            dy = 3 * p + dyo
            if dy > 10:
                continue
            for dx in range(11):
                for c in range(3):
                    lw1[dyo * 33 + dx * 3 + c, p * 64:(p + 1) * 64] = w1[:, c, dy, dx]
    lw1[99, 0:64] = b1
    # conv2: [128, (dy,j,m)]: j<3 -> rows s*64+c = ch c at dx=2j+s; j=3 -> dx=6
    lw2 = np.zeros((128, 7 * 4 * 192), f)
    for dy in range(7):
        for j in range(3):
            for s in range(2):
                lw2[s * 64:(s + 1) * 64, (dy * 4 + j) * 192:(dy * 4 + j + 1) * 192] = \
                    w2[:, :, dy, 2 * j + s].T
        lw2[0:64, (dy * 4 + 3) * 192:(dy * 4 + 4) * 192] = w2[:, :, dy, 6].T
    lb2 = np.zeros((128, 2), f)
    lb2[:, 0] = b2[0:128]
    lb2[0:64, 1] = b2[128:192]
    # conv3: [128, 14592]: cols blk*384+m (kc0); 9600+(blk//2)*384+m rows
    # 64*(blk%2) (kc1)
    lw3 = np.zeros((128, 14592), f)
    for dy in range(5):
        for dx in range(5):
            blk = dy * 5 + dx
            lw3[:, blk * 384:(blk + 1) * 384] = w3[:, 0:128, dy, dx].T
            po = 64 * (blk % 2)
            co = 9600 + (blk // 2) * 384
            lw3[po:po + 64, co:co + 384] = w3[:, 128:192, dy, dx].T
    lb3 = np.zeros((128, 3), f)
    lb3[:, 0] = b3[0:128]; lb3[:, 1] = b3[128:256]; lb3[:, 2] = b3[256:384]
    # conv4 / conv5: [128, (o, m)] with o = (dy*3+dx)*nkc + kc
    lw4 = np.zeros((128, 27 * 256), f)
    for dy in range(3):
        for dx in range(3):
            for kc in range(3):
                o = (dy * 3 + dx) * 3 + kc
                lw4[:, o * 256:(o + 1) * 256] = w4[:, kc * 128:(kc + 1) * 128, dy, dx].T
    lb4 = np.stack([b4[0:128], b4[128:256]], axis=1).astype(f)
    lw5 = np.zeros((128, 18 * 256), f)
    for dy in range(3):
        for dx in range(3):
            for kc in range(2):
                o = (dy * 3 + dx) * 2 + kc
                lw5[:, o * 256:(o + 1) * 256] = w5[:, kc * 128:(kc + 1) * 128, dy, dx].T
    lb5 = np.stack([b5[0:128], b5[128:256]], axis=1).astype(f)
    return dict(lw1=lw1, lw2=lw2, lb2=lb2, lw3=lw3, lb3=lb3,
                lw4=lw4, lb4=lb4, lw5=lw5, lb5=lb5)


def _prep_x13(x):
    """x [B,3,32,32] -> per-core [4, 100, 8*41*32] im2col-packed."""
    f = np.float32
    xpad = np.zeros((B, 3, 44, 42), f)
    xpad[:, :, 5:37, 5:37] = x
    X = np.zeros((100, B, 41, 32), f)
    for dyo in range(3):
        for dx in range(11):
            for c in range(3):
                X[dyo * 33 + dx * 3 + c] = xpad[:, c, dyo:dyo + 41, dx:dx + 32]
    X[99] = 1.0
    out = []
    for r in range(N_CORES):
        pc = X[:, r * BC:(r + 1) * BC]  # [100, 32, 41, 32]
        pc = pc.reshape(100, 4, 8 * 41 * 32).transpose(1, 0, 2)
        out.append(np.ascontiguousarray(pc))
    return out


def _prep_fc(fw1, fb1, fw2, fb2, fw3, fb3):
    f = np.float32
    outs = []
    for r in range(N_CORES):
        sl = slice(512 * r, 512 * (r + 1))
        # fw1s [128, (yx, cc, m)]: fw1[512r+m, (cc*128+k)*16+yx]
        fw1s = fw1[sl].reshape(512, 2, 128, 16).transpose(2, 3, 1, 0).reshape(128, -1)
        fb1s = fb1[sl].reshape(4, 128).T
        # fw2s [128, (kc, m)]: fw2[512r+m, kc*128+k]
        fw2s = fw2[sl].reshape(512, 32, 128).transpose(2, 1, 0).reshape(128, -1)
        fb2s = fb2[sl].reshape(4, 128).T
        # fw3s [128, (kc, m)]: fw3[m, 512r + kc*128 + k]
        fw3s = fw3[:, sl].reshape(100, 4, 128).transpose(2, 1, 0).reshape(128, -1)
        fb3s = (fb3 / N_CORES).reshape(100, 1)
        outs.append(dict(
            fw1s=np.ascontiguousarray(fw1s.astype(f)),
            fb1s=np.ascontiguousarray(fb1s.astype(f)),
            fw2s=np.ascontiguousarray(fw2s.astype(f)),
            fb2s=np.ascontiguousarray(fb2s.astype(f)),
            fw3s=np.ascontiguousarray(fw3s.astype(f)),
            fb3s=np.ascontiguousarray(fb3s.astype(f)),
        ))
    return outs


_CACHE = {}

_SHAPES = dict(
    x13=(4, 100, 8 * 41 * 32), lw1=(100, 4 * 64),
    lw2=(128, 7 * 4 * 192), lb2=(128, 2),
    lw3=(128, 14592), lb3=(128, 3),
    lw4=(128, 27 * 256), lb4=(128, 2),
    lw5=(128, 18 * 256), lb5=(128, 2),
    fw1s=(128, 32 * 512), fb1s=(128, 4),
    fw2s=(128, 32 * 512), fb2s=(128, 4),
    fw3s=(128, 4 * 100), fb3s=(100, 1),
)


def _build():
    if "nc" in _CACHE:
        return _CACHE["nc"]
    nc = bacc.Bacc("TRN2", target_bir_lowering=False, debug=False,
                   num_devices=N_CORES)
    _F32R_INPUTS = {"x13", "lw1", "lw2", "lw3", "lw4", "lw5",
                    "fw1s", "fw2s", "fw3s"}
    t = {name: nc.dram_tensor(
            name, list(shape), F32R if name in _F32R_INPUTS else F32,
            kind="ExternalInput").ap()
         for name, shape in _SHAPES.items()}
    yout = nc.dram_tensor("yout", [100, B], F32, kind="ExternalOutput").ap()
    with tile.TileContext(nc) as tc:
        _emit(nc, tc, t, yout)
    nc.compile()
    _CACHE["nc"] = nc
    return nc


def kernel(x, w1, b1, w2, b2, w3, b3, w4, b4, w5, b5,
           fw1, fb1, fw2, fb2, fw3, fb3):
    args = [np.asarray(a, np.float32) for a in
            (x, w1, b1, w2, b2, w3, b3, w4, b4, w5, b5, fw1, fb1, fw2, fb2, fw3, fb3)]
    (x, w1, b1, w2, b2, w3, b3, w4, b4, w5, b5,
     fw1, fb1, fw2, fb2, fw3, fb3) = args
    nc = _build()
    shared = _prep_shared(w1, b1, w2, b2, w3, b3, w4, b4, w5, b5)
    x13s = _prep_x13(x)
    fcs = _prep_fc(fw1, fb1, fw2, fb2, fw3, fb3)
    in_maps = [{**shared, "x13": x13s[r], **fcs[r]} for r in range(N_CORES)]
    res = run_bass_kernel_spmd(nc, in_maps, list(range(N_CORES)))
    y = res.results[0]["yout"]  # [100, 256]
    return np.ascontiguousarray(y.T)



# revision 3
# speedup vs baseline: 1.3960x; 1.3960x over previous
"""AlexNet-style CNN forward pass on 8 Trainium2 NeuronCores.

Strategy:
  - Convs data-parallel: batch 256 -> 32 per core, channels on partitions,
    conv = sum of shifted matmuls over kernel offsets (weights replicated).
  - conv1 (cin=3) uses host-packed im2col rows (3 dy-offsets x 11 dx x 3 ch
    + ones row for fused bias -> K=100) so the PE array is well utilized.
  - conv2 uses an x-shifted duplicate of its input (K=128 = 2 dx-offsets
    x 64 ch) to fill the contraction dim.
  - FC layers model-parallel: each core owns 512 rows of fc1/fc2 and 512
    K-columns of fc3; activations are AllGathered between layers, fc3
    partials AllReduced.  This cuts per-core FC weight DMA 8x.
  - Matmuls/activations run in bf16 (halves DMA + PE power so the clock
    stays unthrottled); PSUM accumulation + biases + fc3 AllReduce in fp32.
"""

import numpy as np
import ml_dtypes

BF = ml_dtypes.bfloat16

import concourse.bass as bass
import concourse.mybir as mybir
import concourse.tile as tile
from concourse import bacc
from concourse.bass_utils import run_bass_kernel_spmd

N_CORES = 8
B = 256
BC = B // N_CORES  # 32 images per core

F32 = mybir.dt.float32
BF16 = mybir.dt.bfloat16
RELU = mybir.ActivationFunctionType.Relu
IDENT = mybir.ActivationFunctionType.Identity


def _emit(nc, tc, t, yout):
    """Emit the whole network. t: dict name -> DRAM AP."""
    sync = nc.sync
    act = nc.scalar
    dve = nc.vector
    pool_e = nc.gpsimd

    psum = tc.alloc_tile_pool(name="psum", bufs=6, space="PSUM")
    scr = tc.alloc_tile_pool(name="scr", bufs=1, side="left")
    dram = tc.alloc_tile_pool(name="dram", bufs=1, space="DRAM")

    # ---------------- phase pools (queue alloc mode handles overlap) ----
    p_w12 = tc.alloc_tile_pool(name="p_w12", bufs=1, side="left")
    p_x2s = tc.alloc_tile_pool(name="p_x2s", bufs=1, side="left")
    p_x13 = tc.alloc_tile_pool(name="p_x13", bufs=2, side="left")

    # conv1+conv2 weights (host arrays already in SBUF layout)
    lw1 = p_w12.tile([100, 4 * 64], BF16)
    sync.dma_start(lw1[:], t["lw1"][:])
    lw2 = p_w12.tile([128, 7 * 4 * 192], BF16)
    sync.dma_start(lw2[:], t["lw2"][:])
    lb2 = p_w12.tile([128, 2], F32)
    sync.dma_start(lb2[:], t["lb2"][:])

    # conv2 input: [128, BC, 22, 23]; rows 0:64 ch c at x, rows 64:128 ch c at x+1
    X2s = p_x2s.tile([128, BC * 22 * 23], BF16)
    pool_e.memset(X2s[:], 0.0)

    def x2v(p0, p1, b0, nb, y0, ny, x0, nx):
        return X2s[p0:p1].rearrange("p (b y x) -> p b y x", b=BC, y=22, x=23)[
            :, b0:b0 + nb, y0:y0 + ny, x0:x0 + nx]

    # ---------------- conv1 + pool1 ----------------
    _sid = nc.enter_named_scope("L1_conv1", False)[0]
    for bg in range(4):  # groups of 8 images
        xt = p_x13.tile([100, 8 * 41 * 32], BF16, tag="x13")
        sync.dma_start(xt[:], t["x13"][bg])
        xtv = xt.rearrange("k (b y x) -> k b y x", b=8, y=41, x=32)
        for bl in range(8):
            b = bg * 8 + bl
            for h in range(2):  # vertical half of the 32x32 output
                ps = psum.tile([64, 512], F32, tag="ps")
                psv = ps.rearrange("m (y x) -> m y x", y=16, x=32)
                for pi, p in enumerate((0, 3, 6, 9)):
                    nc.tensor.matmul(
                        ps[:],
                        lw1[:, pi * 64:(pi + 1) * 64],
                        xtv[:, bl, h * 16 + p:h * 16 + p + 16, :],
                        start=(pi == 0), stop=(pi == 3),
                    )
                # evict+relu (bias came in via the ones-row), then 2x2 maxpool
                s1 = scr.tile([128, 512], BF16, tag="ev", bufs=3)
                act.activation(s1[0:64, :], ps[:], RELU)
                s1v = s1[0:64, :].rearrange("m (y x) -> m y x", y=16, x=32)
                m1 = scr.tile([64, 128], BF16, tag="m1", bufs=2)
                m2 = scr.tile([64, 128], BF16, tag="m2", bufs=2)
                dve.tensor_max(m1[:], s1v[:, 0::2, 0::2], s1v[:, 0::2, 1::2])
                dve.tensor_max(m2[:], s1v[:, 1::2, 0::2], s1v[:, 1::2, 1::2])
                y0 = h * 8 + 3
                dve.tensor_max(
                    x2v(0, 64, b, 1, y0, 8, 3, 16)[:, 0],
                    m1.rearrange("m (y x) -> m y x", y=8, x=16),
                    m2.rearrange("m (y x) -> m y x", y=8, x=16))
        # duplicate this image-group into the x+1-shifted partition block
        # (engines cannot shift partitions; DMA can)
        sync.dma_start(x2v(64, 128, bg * 8, 8, 0, 22, 0, 22),
                       x2v(0, 64, bg * 8, 8, 0, 22, 1, 22))
    p_x13.release()
    nc.leave_named_scope("L1_conv1", _sid, False)

    # conv3 weights (prefetch during conv2) + conv3 input buffers
    p_w3 = tc.alloc_tile_pool(name="p_w3", bufs=1, side="right")
    p_x3 = tc.alloc_tile_pool(name="p_x3", bufs=1, side="right")
    lw3 = p_w3.tile([128, 14592], BF16)
    sync.dma_start(lw3[:], t["lw3"][:])
    lb3 = p_w3.tile([128, 3], F32)
    sync.dma_start(lb3[:], t["lb3"][:])
    X3a = p_x3.tile([128, BC * 12 * 12], BF16)
    # X3b rows 64:128 duplicate rows 0:64 so kc1 matmuls can run at
    # lhsT base_partition 64 (lw3 packs two kernel offsets per column block)
    X3b = p_x3.tile([128, BC * 12 * 12], BF16)
    pool_e.memset(X3a[:], 0.0)
    pool_e.memset(X3b[:], 0.0)

    def x3v(xab, p0, p1, b0, nb, y0, ny, x0, nx):
        return xab[p0:p1].rearrange("p (b y x) -> p b y x", b=BC, y=12, x=12)[
            :, b0:b0 + nb, y0:y0 + ny, x0:x0 + nx]

    # ---------------- conv2 + pool2 ----------------
    _sid = nc.enter_named_scope("L2_conv2", False)[0]
    lw2v = lw2.rearrange("k (a j m) -> k a j m", a=7, j=4, m=192)
    for nt in range(16):  # pairs of images
        for mc in range(2):
            M = 128 if mc == 0 else 64
            ps = psum.tile([M, 512], F32, tag="ps")
            first = True
            for dy in range(7):
                for j in range(4):
                    K = 128 if j < 3 else 64
                    xoff = 2 * j if j < 3 else 6
                    nc.tensor.matmul(
                        ps[:],
                        lw2v[0:K, dy, j, mc * 128:mc * 128 + M],
                        x2v(0, K, nt * 2, 2, dy, 16, xoff, 16),
                        start=first, stop=(dy == 6 and j == 3),
                    )
                    first = False
            s2 = scr.tile([128, 512], BF16, tag="ev", bufs=3)
            act.activation(s2[:M], ps[:], RELU, bias=lb2[0:M, mc:mc + 1])
            s2v = s2.rearrange("m (b y x) -> m b y x", b=2, y=16, x=16)
            m1 = scr.tile([128, 128], BF16, tag="m1", bufs=2)
            m2 = scr.tile([128, 128], BF16, tag="m2", bufs=2)
            dve.tensor_max(m1[:M], s2v[:M, :, 0::2, 0::2], s2v[:M, :, 0::2, 1::2])
            dve.tensor_max(m2[:M], s2v[:M, :, 1::2, 0::2], s2v[:M, :, 1::2, 1::2])
            m1v = m1.rearrange("m (b y x) -> m b y x", b=2, y=8, x=8)
            m2v = m2.rearrange("m (b y x) -> m b y x", b=2, y=8, x=8)
            if mc == 0:
                dve.tensor_max(x3v(X3a, 0, 128, nt * 2, 2, 2, 8, 2, 8), m1v[:], m2v[:])
            else:
                dve.tensor_max(x3v(X3b, 0, 64, nt * 2, 2, 2, 8, 2, 8), m1v[:64], m2v[:64])
    for g in range(4):  # duplicate X3b into partitions 64:128
        sync.dma_start(x3v(X3b, 64, 128, g * 8, 8, 0, 12, 0, 12),
                       x3v(X3b, 0, 64, g * 8, 8, 0, 12, 0, 12))
    nc.leave_named_scope("L2_conv2", _sid, False)
    p_x2s.release()
    p_w12.release()

    # conv4/5 weights (prefetch during conv3) + conv4 input buffers
    p_w45 = tc.alloc_tile_pool(name="p_w45", bufs=1, side="left")
    p_x4 = tc.alloc_tile_pool(name="p_x4", bufs=1, side="left")
    lw4 = p_w45.tile([128, 27 * 256], BF16)
    sync.dma_start(lw4[:], t["lw4"][:])
    lb4 = p_w45.tile([128, 2], F32)
    sync.dma_start(lb4[:], t["lb4"][:])
    lw5 = p_w45.tile([128, 18 * 256], BF16)
    sync.dma_start(lw5[:], t["lw5"][:])
    lb5 = p_w45.tile([128, 2], F32)
    sync.dma_start(lb5[:], t["lb5"][:])
    X4 = []
    for i in range(3):
        X4.append(p_x4.tile([128, BC * 10 * 10], BF16, name=f"X4_{i}"))
        pool_e.memset(X4[i][:], 0.0)

    def xv10(xab, p0, p1, b0, nb, y0, ny, x0, nx):
        return xab[p0:p1].rearrange("p (b y x) -> p b y x", b=BC, y=10, x=10)[
            :, b0:b0 + nb, y0:y0 + ny, x0:x0 + nx]

    _sid = nc.enter_named_scope("L3_conv3", False)[0]
    # ---------------- conv3 ----------------
    for nt in range(4):  # 8 images
        for mc in range(3):
            ps = psum.tile([128, 512], F32, tag="ps")
            first = True
            for dy in range(5):
                for dx in range(5):
                    blk = dy * 5 + dx
                    nc.tensor.matmul(
                        ps[:],
                        lw3[0:128, blk * 384 + mc * 128:blk * 384 + mc * 128 + 128],
                        x3v(X3a, 0, 128, nt * 8, 8, dy, 8, dx, 8),
                        start=first, stop=False,
                    )
                    first = False
                    po = 64 * (blk % 2)
                    co = 9600 + (blk // 2) * 384
                    nc.tensor.matmul(
                        ps[:],
                        lw3[po:po + 64, co + mc * 128:co + mc * 128 + 128],
                        x3v(X3b, po, po + 64, nt * 8, 8, dy, 8, dx, 8),
                        start=False, stop=(dy == 4 and dx == 4),
                    )
            act.activation(
                xv10(X4[mc], 0, 128, nt * 8, 8, 1, 8, 1, 8),
                ps.rearrange("m (b y x) -> m b y x", b=8, y=8, x=8),
                RELU, bias=lb3[:, mc:mc + 1])
    nc.leave_named_scope("L3_conv3", _sid, False)
    p_x3.release()
    p_w3.release()

    # fc1 weights (prefetch during conv4) + conv5 input buffers
    p_fw1 = tc.alloc_tile_pool(name="p_fw1", bufs=1, side="right")
    p_x5 = tc.alloc_tile_pool(name="p_x5", bufs=1, side="right")
    fw1 = p_fw1.tile([128, 32 * 512], BF16)
    sync.dma_start(fw1[:], t["fw1s"][:])
    fb1 = p_fw1.tile([128, 4], F32)
    sync.dma_start(fb1[:], t["fb1s"][:])
    X5 = []
    for i in range(2):
        X5.append(p_x5.tile([128, BC * 10 * 10], BF16, name=f"X5_{i}"))
        pool_e.memset(X5[i][:], 0.0)

    _sid = nc.enter_named_scope("L4_conv4", False)[0]
    # ---------------- conv4 ----------------
    lw4v = lw4.rearrange("k (o m) -> k o m", o=27)
    for nt in range(4):
        for mc in range(2):
            ps = psum.tile([128, 512], F32, tag="ps")
            first = True
            for dy in range(3):
                for dx in range(3):
                    for kc in range(3):
                        o = (dy * 3 + dx) * 3 + kc
                        nc.tensor.matmul(
                            ps[:],
                            lw4v[:, o, mc * 128:mc * 128 + 128],
                            xv10(X4[kc], 0, 128, nt * 8, 8, dy, 8, dx, 8),
                            start=first, stop=(o == 26),
                        )
                        first = False
            act.activation(
                xv10(X5[mc], 0, 128, nt * 8, 8, 1, 8, 1, 8),
                ps.rearrange("m (b y x) -> m b y x", b=8, y=8, x=8),
                RELU, bias=lb4[:, mc:mc + 1])
    nc.leave_named_scope("L4_conv4", _sid, False)
    p_x4.release()

    # pool5 output
    p_p5 = tc.alloc_tile_pool(name="p_p5", bufs=1, side="left")
    P5 = [p_p5.tile([128, BC * 16], BF16, name=f"P5_{i}") for i in range(2)]

    _sid = nc.enter_named_scope("L5_conv5", False)[0]
    # ---------------- conv5 + pool5 ----------------
    lw5v = lw5.rearrange("k (o m) -> k o m", o=18)
    for nt in range(4):
        for mc in range(2):
            ps = psum.tile([128, 512], F32, tag="ps")
            first = True
            for dy in range(3):
                for dx in range(3):
                    for kc in range(2):
                        o = (dy * 3 + dx) * 2 + kc
                        nc.tensor.matmul(
                            ps[:],
                            lw5v[:, o, mc * 128:mc * 128 + 128],
                            xv10(X5[kc], 0, 128, nt * 8, 8, dy, 8, dx, 8),
                            start=first, stop=(o == 17),
                        )
                        first = False
            s5 = scr.tile([128, 512], BF16, tag="ev", bufs=3)
            act.activation(s5[:], ps[:], RELU, bias=lb5[:, mc:mc + 1])
            s5v = s5.rearrange("m (b y x) -> m b y x", b=8, y=8, x=8)
            m1 = scr.tile([128, 128], BF16, tag="m1", bufs=2)
            m2 = scr.tile([128, 128], BF16, tag="m2", bufs=2)
            dve.tensor_max(m1[:], s5v[:, :, 0::2, 0::2], s5v[:, :, 0::2, 1::2])
            dve.tensor_max(m2[:], s5v[:, :, 1::2, 0::2], s5v[:, :, 1::2, 1::2])
            p5v = P5[mc].rearrange("p (b y x) -> p b y x", b=BC, y=4, x=4)
            dve.tensor_max(
                p5v[:, nt * 8:nt * 8 + 8, :, :],
                m1.rearrange("m (b y x) -> m b y x", b=8, y=4, x=4),
                m2.rearrange("m (b y x) -> m b y x", b=8, y=4, x=4))
    nc.leave_named_scope("L5_conv5", _sid, False)
    # stage pool5 out to DRAM, then free conv-era pools (LIFO per side)
    cin5 = dram.tile([2, 128, BC * 16], BF16)
    sync.dma_start(cin5[0], P5[0][:])
    sync.dma_start(cin5[1], P5[1][:])
    p_x5.release()
    p_p5.release()
    p_w45.release()

    # fc2/fc3 weights (DMA overlaps the gather + fc1)
    p_fw2 = tc.alloc_tile_pool(name="p_fw2", bufs=1, side="left")
    fw2 = p_fw2.tile([128, 32 * 512], BF16)
    sync.dma_start(fw2[:], t["fw2s"][:])
    fb2 = p_fw2.tile([128, 4], F32)
    sync.dma_start(fb2[:], t["fb2s"][:])
    fw3 = p_fw2.tile([128, 4 * 100], BF16)
    sync.dma_start(fw3[:], t["fw3s"][:])
    fb3 = p_fw2.tile([100, 1], F32)
    sync.dma_start(fb3[:], t["fb3s"][:])

    _sid = nc.enter_named_scope("G1_gather", False)[0]
    # ---------------- AllGather pool5 -> fc input ----------------
    g1 = dram.tile([N_CORES, 2, 128, BC * 16], BF16)
    pool_e.collective_compute(
        "AllGather", mybir.AluOpType.bypass,
        replica_groups=[list(range(N_CORES))],
        ins=[cin5.opt()], outs=[g1.opt()])

    p_h1 = tc.alloc_tile_pool(name="p_h1", bufs=1, side="right")
    H1 = [p_h1.tile([128, N_CORES * BC * 16], BF16, name=f"H1_{i}") for i in range(2)]
    for cc in range(2):
        sync.dma_start(
            H1[cc].rearrange("c (r f) -> c r f", r=N_CORES),
            g1[:, cc].rearrange("r c f -> c r f"))

    nc.leave_named_scope("G1_gather", _sid, False)
    _sid = nc.enter_named_scope("F1_fc1", False)[0]
    # ---------------- fc1 (model-parallel over 512 outputs) ----------------
    p_f1 = tc.alloc_tile_pool(name="p_f1", bufs=1, side="left")
    F1 = p_f1.tile([128, 4 * B], BF16)
    fw1v = fw1.rearrange("k (y c m) -> k y c m", y=16, c=2, m=512)
    for mc in range(4):
        ps = psum.tile([128, B], F32, tag="ps")
        first = True
        for yx in range(16):
            for cc in range(2):
                rhs = H1[cc].rearrange("c (r b y) -> c y r b", r=N_CORES, b=BC, y=16)
                nc.tensor.matmul(
                    ps[:],
                    fw1v[:, yx, cc, mc * 128:mc * 128 + 128],
                    rhs[:, yx],
                    start=first, stop=(yx == 15 and cc == 1))
                first = False
        act.activation(F1[:, mc * B:(mc + 1) * B], ps[:], RELU, bias=fb1[:, mc:mc + 1])
    p_h1.release()
    p_fw1.release()

    nc.leave_named_scope("F1_fc1", _sid, False)
    _sid = nc.enter_named_scope("G2_gather", False)[0]
    # ---------------- AllGather fc1 ----------------
    cin6 = dram.tile([128, 4 * B], BF16)
    sync.dma_start(cin6[:], F1[:])
    g2 = dram.tile([N_CORES, 128, 4 * B], BF16)
    pool_e.collective_compute(
        "AllGather", mybir.AluOpType.bypass,
        replica_groups=[list(range(N_CORES))],
        ins=[cin6.opt()], outs=[g2.opt()])
    p_f1.release()

    p_h2 = tc.alloc_tile_pool(name="p_h2", bufs=1, side="right")
    H2 = p_h2.tile([128, N_CORES * 4 * B], BF16)
    sync.dma_start(
        H2.rearrange("c (r f) -> c r f", r=N_CORES),
        g2.rearrange("r c f -> c r f"))

    nc.leave_named_scope("G2_gather", _sid, False)
    _sid = nc.enter_named_scope("F2_fc2", False)[0]
    # ---------------- fc2 ----------------
    p_f2 = tc.alloc_tile_pool(name="p_f2", bufs=1, side="left")
    F2 = p_f2.tile([128, 4 * B], BF16)
    fw2v = fw2.rearrange("k (a m) -> k a m", a=32)
    for mc in range(4):
        ps = psum.tile([128, B], F32, tag="ps")
        for kc in range(32):
            nc.tensor.matmul(
                ps[:], fw2v[:, kc, mc * 128:mc * 128 + 128],
                H2[:, kc * B:(kc + 1) * B],
                start=(kc == 0), stop=(kc == 31))
        act.activation(F2[:, mc * B:(mc + 1) * B], ps[:], RELU, bias=fb2[:, mc:mc + 1])
    p_h2.release()

    nc.leave_named_scope("F2_fc2", _sid, False)
    _sid = nc.enter_named_scope("F3_fc3", False)[0]
    # ---------------- fc3 (partial over this core's 512 K) + AllReduce ----
    fw3v = fw3.rearrange("k (a m) -> k a m", a=4)
    ps = psum.tile([128, B], F32, tag="ps")
    for kc in range(4):
        nc.tensor.matmul(
            ps[0:100, :], fw3v[:, kc, :], F2[:, kc * B:(kc + 1) * B],
            start=(kc == 0), stop=(kc == 3))
    s3 = scr.tile([128, 512], F32, tag="ev", bufs=3)
    act.activation(s3[0:100, 0:B], ps[0:100, :], IDENT, bias=fb3[:])  # + fb3/8
    cin7 = dram.tile([100, B], F32)
    sync.dma_start(cin7[:], s3[0:100, 0:B])
    g3 = dram.tile([100, B], F32)
    pool_e.collective_compute(
        "AllReduce", mybir.AluOpType.add,
        replica_groups=[list(range(N_CORES))],
        ins=[cin7.opt()], outs=[g3.opt()])
    sync.dma_start(yout[:], g3[:])
    nc.leave_named_scope("F3_fc3", _sid, False)
    p_f2.release()
    p_fw2.release()

    scr.release()
    dram.release()
    psum.release()


# ---------------------------------------------------------------------------
# host-side input prep (numpy; all weight arrays already in SBUF layout)
# ---------------------------------------------------------------------------

def _prep_shared(w1, b1, w2, b2, w3, b3, w4, b4, w5, b5):
    f = np.float32
    # conv1: rows r = dyo*33 + dx*3 + c, row 99 = bias(ones); 4 passes dy=3p+dyo
    lw1 = np.zeros((100, 4 * 64), f)
    for p in range(4):
        for dyo in range(3):
            dy = 3 * p + dyo
            if dy > 10:
                continue
            for dx in range(11):
                for c in range(3):
                    lw1[dyo * 33 + dx * 3 + c, p * 64:(p + 1) * 64] = w1[:, c, dy, dx]
    lw1[99, 0:64] = b1
    # conv2: [128, (dy,j,m)]: j<3 -> rows s*64+c = ch c at dx=2j+s; j=3 -> dx=6
    lw2 = np.zeros((128, 7 * 4 * 192), f)
    for dy in range(7):
        for j in range(3):
            for s in range(2):
                lw2[s * 64:(s + 1) * 64, (dy * 4 + j) * 192:(dy * 4 + j + 1) * 192] = \
                    w2[:, :, dy, 2 * j + s].T
        lw2[0:64, (dy * 4 + 3) * 192:(dy * 4 + 4) * 192] = w2[:, :, dy, 6].T
    lb2 = np.zeros((128, 2), f)
    lb2[:, 0] = b2[0:128]
    lb2[0:64, 1] = b2[128:192]
    # conv3: [128, 14592]: cols blk*384+m (kc0); 9600+(blk//2)*384+m rows
    # 64*(blk%2) (kc1)
    lw3 = np.zeros((128, 14592), f)
    for dy in range(5):
        for dx in range(5):
            blk = dy * 5 + dx
            lw3[:, blk * 384:(blk + 1) * 384] = w3[:, 0:128, dy, dx].T
            po = 64 * (blk % 2)
            co = 9600 + (blk // 2) * 384
            lw3[po:po + 64, co:co + 384] = w3[:, 128:192, dy, dx].T
    lb3 = np.zeros((128, 3), f)
    lb3[:, 0] = b3[0:128]; lb3[:, 1] = b3[128:256]; lb3[:, 2] = b3[256:384]
    # conv4 / conv5: [128, (o, m)] with o = (dy*3+dx)*nkc + kc
    lw4 = np.zeros((128, 27 * 256), f)
    for dy in range(3):
        for dx in range(3):
            for kc in range(3):
                o = (dy * 3 + dx) * 3 + kc
                lw4[:, o * 256:(o + 1) * 256] = w4[:, kc * 128:(kc + 1) * 128, dy, dx].T
    lb4 = np.stack([b4[0:128], b4[128:256]], axis=1).astype(f)
    lw5 = np.zeros((128, 18 * 256), f)
    for dy in range(3):
        for dx in range(3):
            for kc in range(2):
                o = (dy * 3 + dx) * 2 + kc
                lw5[:, o * 256:(o + 1) * 256] = w5[:, kc * 128:(kc + 1) * 128, dy, dx].T
    lb5 = np.stack([b5[0:128], b5[128:256]], axis=1).astype(f)
    return dict(lw1=lw1.astype(BF), lw2=lw2.astype(BF), lb2=lb2,
                lw3=lw3.astype(BF), lb3=lb3, lw4=lw4.astype(BF), lb4=lb4,
                lw5=lw5.astype(BF), lb5=lb5)


def _prep_x13(x):
    """x [B,3,32,32] -> per-core [4, 100, 8*41*32] im2col-packed."""
    f = np.float32
    xpad = np.zeros((B, 3, 44, 42), f)
    xpad[:, :, 5:37, 5:37] = x
    X = np.zeros((100, B, 41, 32), f)
    for dyo in range(3):
        for dx in range(11):
            for c in range(3):
                X[dyo * 33 + dx * 3 + c] = xpad[:, c, dyo:dyo + 41, dx:dx + 32]
    X[99] = 1.0
    out = []
    for r in range(N_CORES):
        pc = X[:, r * BC:(r + 1) * BC]  # [100, 32, 41, 32]
        pc = pc.reshape(100, 4, 8 * 41 * 32).transpose(1, 0, 2)
        out.append(np.ascontiguousarray(pc).astype(BF))
    return out


def _prep_fc(fw1, fb1, fw2, fb2, fw3, fb3):
    f = np.float32
    outs = []
    for r in range(N_CORES):
        sl = slice(512 * r, 512 * (r + 1))
        # fw1s [128, (yx, cc, m)]: fw1[512r+m, (cc*128+k)*16+yx]
        fw1s = fw1[sl].reshape(512, 2, 128, 16).transpose(2, 3, 1, 0).reshape(128, -1)
        fb1s = fb1[sl].reshape(4, 128).T
        # fw2s [128, (kc, m)]: fw2[512r+m, kc*128+k]
        fw2s = fw2[sl].reshape(512, 32, 128).transpose(2, 1, 0).reshape(128, -1)
        fb2s = fb2[sl].reshape(4, 128).T
        # fw3s [128, (kc, m)]: fw3[m, 512r + kc*128 + k]
        fw3s = fw3[:, sl].reshape(100, 4, 128).transpose(2, 1, 0).reshape(128, -1)
        fb3s = (fb3 / N_CORES).reshape(100, 1)
        outs.append(dict(
            fw1s=np.ascontiguousarray(fw1s).astype(BF),
            fb1s=np.ascontiguousarray(fb1s.astype(f)),
            fw2s=np.ascontiguousarray(fw2s).astype(BF),
            fb2s=np.ascontiguousarray(fb2s.astype(f)),
            fw3s=np.ascontiguousarray(fw3s).astype(BF),
            fb3s=np.ascontiguousarray(fb3s.astype(f)),
        ))
    return outs


_CACHE = {}

_SHAPES = dict(
    x13=(4, 100, 8 * 41 * 32), lw1=(100, 4 * 64),
    lw2=(128, 7 * 4 * 192), lb2=(128, 2),
    lw3=(128, 14592), lb3=(128, 3),
    lw4=(128, 27 * 256), lb4=(128, 2),
    lw5=(128, 18 * 256), lb5=(128, 2),
    fw1s=(128, 32 * 512), fb1s=(128, 4),
    fw2s=(128, 32 * 512), fb2s=(128, 4),
    fw3s=(128, 4 * 100), fb3s=(100, 1),
)


def _build():
    if "nc" in _CACHE:
        return _CACHE["nc"]
    nc = bacc.Bacc("TRN2", target_bir_lowering=False, debug=False,
                   num_devices=N_CORES)
    _BF16_INPUTS = {"x13", "lw1", "lw2", "lw3", "lw4", "lw5",
                    "fw1s", "fw2s", "fw3s"}
    t = {name: nc.dram_tensor(
            name, list(shape), BF16 if name in _BF16_INPUTS else F32,
            kind="ExternalInput").ap()
         for name, shape in _SHAPES.items()}
    yout = nc.dram_tensor("yout", [100, B], F32, kind="ExternalOutput").ap()
    with tile.TileContext(nc) as tc:
        _emit(nc, tc, t, yout)
    nc.compile()
    _CACHE["nc"] = nc
    return nc


def _in_maps(inputs):
    inputs = {k: np.asarray(v, np.float32) for k, v in inputs.items()}
    shared = _prep_shared(*[inputs[k] for k in
                            ("w1", "b1", "w2", "b2", "w3", "b3", "w4", "b4",
                             "w5", "b5")])
    x13s = _prep_x13(inputs["x"])
    fcs = _prep_fc(*[inputs[k] for k in
                     ("fw1", "fb1", "fw2", "fb2", "fw3", "fb3")])
    return [{**shared, "x13": x13s[r], **fcs[r]} for r in range(N_CORES)]


def kernel(x, w1, b1, w2, b2, w3, b3, w4, b4, w5, b5,
           fw1, fb1, fw2, fb2, fw3, fb3):
    nc = _build()
    in_maps = _in_maps(dict(x=x, w1=w1, b1=b1, w2=w2, b2=b2, w3=w3, b3=b3,
                            w4=w4, b4=b4, w5=w5, b5=b5, fw1=fw1, fb1=fb1,
                            fw2=fw2, fb2=fb2, fw3=fw3, fb3=fb3))
    res = run_bass_kernel_spmd(nc, in_maps, list(range(N_CORES)))
    y = res.results[0]["yout"]  # [100, 256]
    return np.ascontiguousarray(y.T)

